# revision 12
# baseline (speedup 1.0000x reference)
"""DGCNN classifier on 8 TRN2 NeuronCores — pure data parallel (1 sample/core).

Per-core algorithm (feature-major [C, N] layout):
  Edge conv block (W = [W1 | W2] acting on [x_j - x_i, x_i], BN folded to s, t):
    out[i] = leaky( max_{j in knn(i)} A'[j] + C'[i] )
    with A' = s * (X @ W1^T),  C' = s * (X @ (W2 - W1)^T) + t
  (valid because leaky_relu is monotone and h[i,j] = A'[j] + C'[i]).
  knn via S[i, j] = 2 x_i.x_j - ||x_j||^2 (row-constant -||x_i||^2 dropped),
  top-20 per row with vector.max / max_index / match_replace (rounds of 8).
  Neighbor rows gathered with gpsimd.dma_gather (wrapped int16 indices),
  reduced with halving tensor_tensor max.

kernel(**inputs) takes the FULL unsharded inputs and returns [B, 40].
"""

import numpy as np

import concourse.mybir as mybir
import concourse.tile as tile
from concourse import bacc
from concourse.bass_utils import run_bass_kernel_spmd
P = 128
KNN = 20
LAYERS = [(3, 64), (64, 64), (64, 128), (128, 256)]  # (c_in, o)
EMB = 1024
NCLS = 40
EPS = 1e-5
SLOPE = 0.2
NEG = -3.0e38

F32 = mybir.dt.float32
F32R = mybir.dt.float32r
U32 = mybir.dt.uint32
I16 = mybir.dt.int16


def build_module(N=1024, use_f32r=True, topk_from_psum=False):
    """Build + compile the per-core Bass module."""
    NB = N // P
    NW = P * KNN // 16  # wrapped idx cols per node block (160)
    MMDT = F32R if use_f32r else F32

    nc = bacc.Bacc("TRN2", target_bir_lowering=False, debug=False)

    # ---- DRAM I/O ----
    xt0_d = nc.dram_tensor("xt0", [3, N], F32, kind="ExternalInput")
    w1_d, w2_d = [], []
    for li, (c, o) in enumerate(LAYERS):
        w1_d.append(nc.dram_tensor(f"w1t{li}", [c, o], F32, kind="ExternalInput"))
        w2_d.append(nc.dram_tensor(f"w2t{li}", [c + 1, o], F32, kind="ExternalInput"))
    wft_d = nc.dram_tensor("wft", [513, EMB], F32, kind="ExternalInput")
    wfc1_d = nc.dram_tensor("wfc1", [2 * EMB + 1, 512], F32, kind="ExternalInput")
    wfc2_d = nc.dram_tensor("wfc2", [513, 256], F32, kind="ExternalInput")
    wfc3_d = nc.dram_tensor("wfc3", [257, NCLS], F32, kind="ExternalInput")
    out_d = nc.dram_tensor("out", [1, NCLS], F32, kind="ExternalOutput")

    with tile.TileContext(nc) as tc:
        with (
            tc.tile_pool(name="sb", bufs=1) as sb,
            tc.tile_pool(name="ps", bufs=1, space="PSUM") as ps,
            tc.tile_pool(name="dr", bufs=1, space="DRAM") as dr,
        ):
            # ---------- persistent tiles ----------
            ones_col = sb.tile([P, 1], F32)
            nc.gpsimd.memset(ones_col[:], 1.0)
            ones_row_f = sb.tile([1, N], F32)
            nc.gpsimd.memset(ones_row_f[:], 1.0)
            ones_row_r = sb.tile([1, N], MMDT)
            nc.vector.tensor_copy(ones_row_r[:], ones_row_f[:])

            # concat (feature-major) x1..x4 -> [512, N] in 4 partition tiles
            tc0 = sb.tile([P, N], F32)  # x1 (rows 0:64) | x2 (rows 64:128)
            tc1 = sb.tile([P, N], F32)  # x3
            tc2 = sb.tile([P, N], F32)  # x4[0:128]
            tc3 = sb.tile([P, N], F32)  # x4[128:256]
            # fp32r twins (tcr1 built at L4 prep, others at fuse)
            tcr = [None, None, None, None]

            # per-layer augmented inputs (features + ones row)
            xt0aug = sb.tile([3, N], F32)
            nc.sync.dma_start(xt0aug[:], xt0_d[:])
            xt1aug = sb.tile([65, N], F32)
            nc.vector.memset(xt1aug[64:65, :], 1.0)
            xt2aug = sb.tile([65, N], F32)
            nc.vector.memset(xt2aug[64:65, :], 1.0)

            # layer descriptors: chunks = list of (src f32 AP, feat_rows, has_ones)
            layer_in = [
                [(xt0aug, 3, False), (None, 0, True)],
                [(xt1aug, 64, True)],
                [(xt2aug, 64, True)],
                [(tc1, 128, False), (None, 0, True)],  # None -> ones_row
            ]
            # pass-2 output destinations per (layer, oc): feature-major tiles
            # (base partition 0 each; cross-partition copies handled at layer end)
            fdest = [
                [xt1aug],
                [xt2aug],
                [tc1],
                [tc2, tc3],
            ]

            for li, (c, o) in enumerate(LAYERS):
                chunks = layer_in[li]

                # ---- weights to SBUF (staged f32, cast to fp32r) ----
                w1_f = sb.tile([c, o], F32, name="w1_f", tag="w1_f", bufs=2)
                nc.sync.dma_start(w1_f[:], w1_d[li][:])
                w2_f = sb.tile([c, o], F32, name="w2_f", tag="w2_f", bufs=2)
                nc.sync.dma_start(w2_f[:], w2_d[li][0:c, :])
                w2b_f = sb.tile([1, o], F32, name="w2b_f", tag="w2b_f", bufs=2)
                nc.sync.dma_start(w2b_f[:], w2_d[li][c : c + 1, :])
                w1_sb = sb.tile([c, o], MMDT, name="w1_sb", tag="w1_sb", bufs=2)
                nc.vector.tensor_copy(w1_sb[:], w1_f[:])
                w2_sb = sb.tile([c, o], MMDT, name="w2_sb", tag="w2_sb", bufs=2)
                nc.vector.tensor_copy(w2_sb[:], w2_f[:])
                w2b_sb = sb.tile([1, o], MMDT, name="w2b_sb", tag="w2b_sb", bufs=2)
                nc.vector.tensor_copy(w2b_sb[:], w2b_f[:])

                # ---- prep: squared norms ----
                sq = sb.tile([P, N], F32, name="sq", tag="sq")
                foff = 0
                for src, fr, _ in chunks:
                    if fr:
                        nc.vector.tensor_tensor(
                            out=sq[foff : foff + fr, :],
                            in0=src[0:fr, :],
                            in1=src[0:fr, :],
                            op=mybir.AluOpType.mult,
                        )
                        foff += fr
                xx0 = ps.tile([1, 512], F32, space="PSUM", name="xx0", tag="misc", bufs=2)
                xx1 = ps.tile([1, 512], F32, space="PSUM", name="xx1", tag="misc", bufs=2)
                for j, xxp in enumerate((xx0, xx1)):
                    nc.tensor.matmul(
                        xxp[:],
                        lhsT=ones_col[0:c, 0:1],
                        rhs=sq[0:c, j * 512 : (j + 1) * 512],
                        start=True,
                        stop=True,
                    )

                # ---- V rows: [2*X ; -xx], staged f32 then rounded ----
                vm_r = []
                for ci, (src, fr, has_ones) in enumerate(chunks):
                    rows = fr + (1 if has_ones else 0)
                    vf = sb.tile([rows, N], F32, name="vm_f", tag="vmf", bufs=2)
                    if fr:
                        nc.vector.tensor_scalar_mul(vf[0:fr, :], src[0:fr, :], 2.0)
                    if has_ones:
                        for j, xxp in enumerate((xx0, xx1)):
                            nc.scalar.activation(
                                vf[fr : fr + 1, j * 512 : (j + 1) * 512],
                                xxp[:],
                                mybir.ActivationFunctionType.Copy,
                                scale=-1.0,
                            )
                    vm_r.append(vf)  # S matmul stays full fp32

                # ---- fp32r stationary twins of the input chunks ----
                xtr = []
                for ci, (src, fr, has_ones) in enumerate(chunks):
                    if src is None:
                        xtr.append(ones_row_r)
                        continue
                    rows = fr + (1 if has_ones else 0)
                    if li == 3 and ci == 0:
                        # tc1 twin doubles as the fuse concat operand
                        tcr[1] = sb.tile([P, N], MMDT, name="tcr1")
                        nc.vector.tensor_copy(tcr[1][:], tc1[:])
                        xtr.append(tcr[1])
                    else:
                        xr = sb.tile([rows, N], MMDT, name="xt_r", tag="xtr", bufs=2)
                        nc.vector.tensor_copy(xr[:], src[0:rows, :])
                        xtr.append(xr)

                # per-layer index tiles (int32-staged wrapped build)
                idx32all = sb.tile(
                    [P, NB * KNN], U32, name="idx32all", tag="idx32", bufs=2
                )
                w32 = sb.tile([16, NB * NW], U32, name="w32", tag="w32", bufs=2)
                widx = sb.tile([P, NB * NW], I16, name="widx", tag="widx", bufs=2)

                # feature-major A' / C' tables [o, N] (OC partition chunks)
                OC = max(1, o // P)
                ow = o // OC
                atabs, ctabs = [], []
                for oc in range(OC):
                    ocs = slice(oc * ow, (oc + 1) * ow)
                    atab = sb.tile([ow, N], F32, name="atab", tag="atab", bufs=2)
                    ctab = sb.tile([ow, N], F32, name="ctab", tag="ctab", bufs=2)
                    fchunks = [
                        (ci, fr) for ci, (_, fr, _) in enumerate(chunks) if fr
                    ]
                    assert len(fchunks) == 1  # features always in one K chunk
                    fci, fr = fchunks[0]
                    for j in range(N // 512):
                        js = slice(j * 512, (j + 1) * 512)
                        ac_ps = ps.tile(
                            [ow, 512], F32, space="PSUM", name="ac_ps",
                            tag="AC", bufs=2,
                        )
                        nc.tensor.matmul(
                            ac_ps[:],
                            lhsT=w1_sb[0:fr, ocs],
                            rhs=xtr[fci][0:fr, js],
                            start=True,
                            stop=True,
                        )
                        nc.scalar.copy(atab[:, js], ac_ps[:])
                        cc_ps = ps.tile(
                            [ow, 512], F32, space="PSUM", name="cc_ps",
                            tag="AC", bufs=2,
                        )
                        nc.tensor.matmul(
                            cc_ps[:],
                            lhsT=w2_sb[0:fr, ocs],
                            rhs=xtr[fci][0:fr, js],
                            start=True,
                            stop=False,
                        )
                        nc.tensor.matmul(
                            cc_ps[:],
                            lhsT=w2b_sb[0:1, ocs],
                            rhs=ones_row_r[0:1, js],
                            start=False,
                            stop=True,
                        )
                        nc.scalar.copy(ctab[:, js], cc_ps[:])
                    atabs.append(atab)
                    ctabs.append(ctab)

                # ---------- pass 1: S matmul, topk, A'/C' ----------
                for ib in range(NB):
                    ibs = slice(ib * P, (ib + 1) * P)
                    s_ps = ps.tile(
                        [P, N], F32, space="PSUM", name="s_ps", tag="S", bufs=2
                    )
                    nchunks = len(chunks)
                    for j in range(N // 512):
                        for ci in range(nchunks):
                            src_f32 = chunks[ci][0]
                            if src_f32 is None:
                                src_f32 = ones_row_f
                            rows = chunks[ci][1] + (1 if chunks[ci][2] else 0)
                            nc.tensor.matmul(
                                s_ps[:, j * 512 : (j + 1) * 512],
                                lhsT=src_f32[0:rows, ibs],
                                rhs=vm_r[ci][0:rows, j * 512 : (j + 1) * 512],
                                start=(ci == 0),
                                stop=(ci == nchunks - 1),
                            )
                    if topk_from_psum:
                        s0 = s_ps
                    else:
                        s0 = sb.tile([P, N], F32, name="s0", tag="s0", bufs=2)
                        nc.scalar.copy(s0[:], s_ps[:])
                    # top-20 (3 rounds of 8)
                    smax = sb.tile([P, 8], F32, name="smax", tag="smax", bufs=2)
                    idx24 = sb.tile([P, 24], U32, name="idx24", tag="idx24", bufs=2)
                    s2 = sb.tile([P, N], F32, name="s2", tag="s2", bufs=1)
                    s3 = sb.tile([P, N], F32, name="s3", tag="s3", bufs=1)
                    nc.vector.max(smax[:], s0[:])
                    nc.vector.max_index(idx24[:, 0:8], smax[:], s0[:])
                    nc.vector.match_replace(s2[:], smax[:], s0[:], NEG)
                    nc.vector.max(smax[:], s2[:])
                    nc.vector.max_index(idx24[:, 8:16], smax[:], s2[:])
                    nc.vector.match_replace(s3[:], smax[:], s2[:], NEG)
                    nc.vector.max(smax[:], s3[:])
                    nc.vector.max_index(idx24[:, 16:24], smax[:], s3[:])
                    nc.vector.tensor_copy(
                        idx32all[:, ib * KNN : (ib + 1) * KNN], idx24[:, 0:KNN]
                    )

                # ---- wrapped idx build (8 strided DMAs, cast, 3 doublings) ----
                for q in range(8):
                    nc.sync.dma_start(
                        w32[0:16, q : NB * NW : 8],
                        idx32all[16 * q : 16 * (q + 1), :],
                    )
                nc.vector.tensor_copy(widx[0:16, :], w32[:])
                nc.sync.dma_start(widx[16:32, :], widx[0:16, :])
                nc.sync.dma_start(widx[32:64, :], widx[0:32, :])
                nc.sync.dma_start(widx[64:128, :], widx[0:64, :])

                # ---------- pass 2: ap_gather, neighbor max, combine ----------
                for ib in range(NB):
                    ibs = slice(ib * P, (ib + 1) * P)
                    eng = nc.vector
                    for oc in range(OC):
                        g = sb.tile([P, KNN * P], F32, name="g", tag="g", bufs=2)
                        nc.gpsimd.ap_gather(
                            out_ap=g[0:ow, :].rearrange("c (t n) -> c t n", t=KNN),
                            in_ap=atabs[oc][:].rearrange("c (n d) -> c n d", d=1),
                            idxs_ap=widx[0:ow, ib * NW : (ib + 1) * NW],
                            channels=ow,
                            num_elems=N,
                            d=1,
                            num_idxs=P * KNN,
                        )
                        m = KNN
                        while m > 1:
                            h = m // 2
                            eng.tensor_tensor(
                                out=g[0:ow, : h * P],
                                in0=g[0:ow, : h * P],
                                in1=g[0:ow, h * P : 2 * h * P],
                                op=mybir.AluOpType.max,
                            )
                            if m % 2:
                                eng.tensor_tensor(
                                    out=g[0:ow, :P],
                                    in0=g[0:ow, :P],
                                    in1=g[0:ow, (m - 1) * P : m * P],
                                    op=mybir.AluOpType.max,
                                )
                            m = h
                        # z = M + C'; out = leaky(z) written straight to dest
                        xn = sb.tile([P, P], F32, name="xn", tag="xn", bufs=2)
                        eng.tensor_tensor(
                            out=xn[0:ow, :],
                            in0=g[0:ow, :P],
                            in1=ctabs[oc][:, ibs],
                            op=mybir.AluOpType.add,
                        )
                        eng.tensor_scalar_mul(g[0:ow, :P], xn[0:ow, :], SLOPE)
                        dst = fdest[li][oc]
                        eng.tensor_tensor(
                            out=dst[0:ow, ibs],
                            in0=xn[0:ow, :],
                            in1=g[0:ow, :P],
                            op=mybir.AluOpType.max,
                        )
                # layer-end fanout into the concat tiles
                if li == 0:
                    nc.scalar.copy(tc0[0:64, :], xt1aug[0:64, :])
                elif li == 1:
                    nc.sync.dma_start(tc0[64:128, :], xt2aug[0:64, :])

            # ---------- fuse: G^T = Wfs_aug @ Xc_aug, leaky, pooling ----------
            wf_sb = sb.tile([P, 4 * EMB], MMDT)
            for kc in range(4):
                wst = sb.tile([P, EMB], F32, name="wst", tag="wst", bufs=2)
                nc.sync.dma_start(wst[:], wft_d[kc * P : (kc + 1) * P, :])
                nc.vector.tensor_copy(wf_sb[:, kc * EMB : (kc + 1) * EMB], wst[:])
            wfb_f = sb.tile([1, EMB], F32)
            nc.sync.dma_start(wfb_f[:], wft_d[512:513, :])
            wfb_sb = sb.tile([1, EMB], MMDT)
            nc.vector.tensor_copy(wfb_sb[:], wfb_f[:])

            for i, t in enumerate((tc0, tc1, tc2, tc3)):
                if tcr[i] is None:
                    tcr[i] = sb.tile([P, N], MMDT, name=f"tcr{i}")
                    nc.vector.tensor_copy(tcr[i][:], t[:])

            fmax = sb.tile([P, 8], F32)
            fsum = sb.tile([P, 8], F32)
            for mi in range(EMB // P):
                gf = sb.tile([P, N], F32, name="gf", tag="g", bufs=2)
                for j in range(N // 512):
                    g_ps = ps.tile(
                        [P, 512], F32, space="PSUM", name="g_ps", tag="misc", bufs=2
                    )
                    for kc in range(4):
                        nc.tensor.matmul(
                            g_ps[:],
                            lhsT=wf_sb[:, kc * EMB + mi * P : kc * EMB + (mi + 1) * P],
                            rhs=tcr[kc][:, j * 512 : (j + 1) * 512],
                            start=(kc == 0),
                            stop=False,
                        )
                    nc.tensor.matmul(
                        g_ps[:],
                        lhsT=wfb_sb[:, mi * P : (mi + 1) * P],
                        rhs=ones_row_r[:, j * 512 : (j + 1) * 512],
                        start=False,
                        stop=True,
                    )
                    eng = nc.vector  # GPSIMD cannot read PSUM
                    js = slice(j * 512, (j + 1) * 512)
                    eng.tensor_scalar_mul(gf[:, js], g_ps[:], SLOPE)
                    eng.tensor_tensor(
                        out=gf[:, js],
                        in0=gf[:, js],
                        in1=g_ps[:],
                        op=mybir.AluOpType.max,
                    )
                nc.vector.reduce_max(
                    fmax[:, mi : mi + 1], gf[:], axis=mybir.AxisListType.X
                )
                nc.vector.reduce_sum(
                    fsum[:, mi : mi + 1], gf[:], axis=mybir.AxisListType.X
                )

            # ---------- fc stack (fp32) ----------
            wfc1b = sb.tile([1, 512], F32)
            nc.sync.dma_start(wfc1b[:], wfc1_d[2 * EMB : 2 * EMB + 1, :])
            wfc2_sb = sb.tile([P, 4 * 256], F32)
            for kc in range(4):
                nc.sync.dma_start(
                    wfc2_sb[:, kc * 256 : (kc + 1) * 256],
                    wfc2_d[kc * P : (kc + 1) * P, :],
                )
            wfc2b = sb.tile([1, 256], F32)
            nc.sync.dma_start(wfc2b[:], wfc2_d[512:513, :])
            wfc3_sb = sb.tile([P, 2 * NCLS], F32)
            for kc in range(2):
                nc.sync.dma_start(
                    wfc3_sb[:, kc * NCLS : (kc + 1) * NCLS],
                    wfc3_d[kc * P : (kc + 1) * P, :],
                )
            wfc3b = sb.tile([1, NCLS], F32)
            nc.sync.dma_start(wfc3b[:], wfc3_d[256:257, :])

            fc1_ps = ps.tile([1, 512], F32, space="PSUM", name="fc1_ps", tag="misc", bufs=2)
            for kc in range(16):
                wc = sb.tile([P, 512], F32, name="wfc1c", tag="g", bufs=2)
                nc.sync.dma_start(wc[:], wfc1_d[kc * P : (kc + 1) * P, :])
                feat_col = fmax if kc < 8 else fsum
                nc.tensor.matmul(
                    fc1_ps[:],
                    lhsT=feat_col[:, kc % 8 : kc % 8 + 1],
                    rhs=wc[:],
                    start=(kc == 0),
                    stop=False,
                )
            nc.tensor.matmul(
                fc1_ps[:], lhsT=ones_col[0:1, 0:1], rhs=wfc1b[:], start=False, stop=True
            )
            fc1_sb = sb.tile([1, 512], F32)
            fc1_tmp = sb.tile([1, 512], F32)
            nc.vector.tensor_scalar_mul(fc1_tmp[:], fc1_ps[:], SLOPE)
            nc.vector.tensor_tensor(
                out=fc1_sb[:], in0=fc1_tmp[:], in1=fc1_ps[:], op=mybir.AluOpType.max
            )
            f1T = sb.tile([P, 4], F32)
            for cc in range(4):
                nc.sync.dma_start(
                    f1T[:, cc : cc + 1], fc1_sb[0:1, cc * P : (cc + 1) * P]
                )

            fc2_ps = ps.tile([1, 256], F32, space="PSUM", name="fc2_ps", tag="misc", bufs=2)
            for kc in range(4):
                nc.tensor.matmul(
                    fc2_ps[:],
                    lhsT=f1T[:, kc : kc + 1],
                    rhs=wfc2_sb[:, kc * 256 : (kc + 1) * 256],
                    start=(kc == 0),
                    stop=False,
                )
            nc.tensor.matmul(
                fc2_ps[:], lhsT=ones_col[0:1, 0:1], rhs=wfc2b[:], start=False, stop=True
            )
            fc2_sb = sb.tile([1, 256], F32)
            fc2_tmp = sb.tile([1, 256], F32)
            nc.vector.tensor_scalar_mul(fc2_tmp[:], fc2_ps[:], SLOPE)
            nc.vector.tensor_tensor(
                out=fc2_sb[:], in0=fc2_tmp[:], in1=fc2_ps[:], op=mybir.AluOpType.max
            )
            f2T = sb.tile([P, 2], F32)
            for cc in range(2):
                nc.sync.dma_start(
                    f2T[:, cc : cc + 1], fc2_sb[0:1, cc * P : (cc + 1) * P]
                )

            fc3_ps = ps.tile([1, NCLS], F32, space="PSUM", name="fc3_ps", tag="misc", bufs=2)
            for kc in range(2):
                nc.tensor.matmul(
                    fc3_ps[:],
                    lhsT=f2T[:, kc : kc + 1],
                    rhs=wfc3_sb[:, kc * NCLS : (kc + 1) * NCLS],
                    start=(kc == 0),
                    stop=False,
                )
            nc.tensor.matmul(
                fc3_ps[:], lhsT=ones_col[0:1, 0:1], rhs=wfc3b[:], start=False, stop=True
            )
            out_sb = sb.tile([1, NCLS], F32)
            nc.scalar.copy(out_sb[:], fc3_ps[:])
            nc.sync.dma_start(out_d[:], out_sb[:])

    nc.compile()
    return nc


# ---------------- host side ----------------

def _np(x):
    return np.asarray(x, dtype=np.float32)


def fold_params(params):
    """Fold BN into weights; build the transposed/augmented host tensors."""
    t = {}
    for li, (c, o) in enumerate(LAYERS):
        p = params[f"ec{li + 1}"]
        W, g, b, m, v = _np(p["W"]), _np(p["g"]), _np(p["b"]), _np(p["m"]), _np(p["v"])
        s = g / np.sqrt(v + EPS)
        tb = b - m * s
        W1 = W[:, :c]
        W2 = W[:, c:]
        t[f"w1t{li}"] = np.ascontiguousarray((s[:, None] * W1).T)  # [c, o]
        w2m = (s[:, None] * (W2 - W1)).T  # [c, o]
        t[f"w2t{li}"] = np.ascontiguousarray(
            np.concatenate([w2m, tb[None, :]], axis=0)
        )  # [c+1, o]
    p = params["fuse"]
    W, g, b, m, v = _np(p["W"]), _np(p["g"]), _np(p["b"]), _np(p["m"]), _np(p["v"])
    s = g / np.sqrt(v + EPS)
    tb = b - m * s
    t["wft"] = np.ascontiguousarray(
        np.concatenate([(s[:, None] * W).T, tb[None, :]], axis=0)
    )  # [513, EMB]
    p = params["fc1"]
    W, g, b, m, v = _np(p["W"]), _np(p["g"]), _np(p["b"]), _np(p["m"]), _np(p["v"])
    s = g / np.sqrt(v + EPS)
    tb = b - m * s
    w = (s[:, None] * W).T.copy()  # [2048, 512]
    w[EMB:, :] *= 1.0 / 1024.0  # fold mean-pool scale (N=1024)
    t["wfc1"] = np.ascontiguousarray(np.concatenate([w, tb[None, :]], axis=0))
    p = params["fc2"]
    W, g, b, m, v = _np(p["W"]), _np(p["g"]), _np(p["b"]), _np(p["m"]), _np(p["v"])
    s = g / np.sqrt(v + EPS)
    tb = b - m * s
    t["wfc2"] = np.ascontiguousarray(
        np.concatenate([(s[:, None] * W).T, tb[None, :]], axis=0)
    )
    p = params["fc3"]
    W, bias = _np(p["W"]), _np(p["bias"])
    t["wfc3"] = np.ascontiguousarray(np.concatenate([W.T, bias[None, :]], axis=0))
    return t


_CACHE = {}


def _run(points, params, k, trace=False):
    points = _np(points)
    B, N, C = points.shape
    assert int(k) == KNN, f"kernel compiled for k={KNN}, got {k}"
    assert N == 1024 and C == 3 and B == 8

    if "nc" not in _CACHE:
        _CACHE["nc"] = build_module(N=N)
    nc = _CACHE["nc"]

    w = fold_params(params)
    in_maps = []
    for core in range(B):
        m = {"xt0": np.ascontiguousarray(points[core].T)}
        m.update(w)
        in_maps.append(m)

    res = run_bass_kernel_spmd(nc, in_maps, core_ids=list(range(B)), trace=trace)
    out = np.concatenate([res.results[i]["out"] for i in range(B)], axis=0)
    return out, res


def kernel(points, params, k):
    return _run(points, params, k)[0]
